# revision 47
# baseline (speedup 1.0000x reference)
"""CDist kernel for Trainium2 (8 NeuronCores, SPMD data-parallel over x rows).

out[i, j] = sqrt(sum_d (x[i,d] - y[j,d])^2),  x: [2048, 64], y: [2048, 64].

Sharding: x rows split 8 ways (256 rows/core), y replicated. Host-side
prep re-lays the inputs into matmul-native [K, N] fp16 operands with both
norm terms folded into an augmented K=66 contraction:
  lhsT = xaT [66, 256]: rows 0..63 = x^T, row 64 = 1, row 65 = -|x_i|^2/2
  rhs  = yaT [66, 2048]: rows 0..63 = y^T, row 64 = -|y_j|^2/2, row 65 = 1
so psum = x.y - |x|^2/2 - |y|^2/2 = -d^2/2 and each [128, 1024] output
chunk is one ScalarE instruction: sqrt(-2 * psum), written as fp16 (the
rel-err budget is 2e-2; fp16 in/out costs ~8e-4 and halves the HBM
traffic that bounds this memory-regime problem).

Device pipeline per core (7 DMAs total; loads xa + ya-half-1 ride the
otherwise-idle Pool SWDGE queue, ya-half-0 and all stores the SP HWDGE
queue — HW-measured fastest split): load xa + two ya halves -> 8 fp16
matmuls ([128,512] each, ordered to chase the ya-half arrivals) -> 4
chunked sqrts on ACT (the serial ~4.2us floor) -> 4 chunked stores that
start as soon as their chunk's sqrt lands. PSUM and output tiles are
per-chunk so the tile-granularity dependency tracker introduces no
false WAR serialization; a small warm matmul keeps the PE HAM
clock-gate ramping during the input DMAs.

The O(N^2) work (matmuls, sqrts, the 8.4 MB of fp16 stores) runs
entirely on device; host marshaling is O(N*D) layout plus the final
fp16 -> fp32 cast of the returned array.
"""

import os

import numpy as np

# Persistent XLA/NEFF compile cache so repeated runs skip recompilation.
os.environ.setdefault("JAX_COMPILATION_CACHE_DIR", "/tmp/jax_comp_cache")

N = 2048
D = 64
N_CORES = 8
ROWS_PER_CORE = N // N_CORES  # 256

K_AUG = D + 2  # 66: data rows + (-|y|^2/2) row + (-|x|^2/2) row
M_TILE = 128
N_TILE = 512
N_MTILES = ROWS_PER_CORE // M_TILE  # 2
N_NTILES = N // N_TILE  # 4
N_HALF = N // 2  # 1024: ya arrives in two half-DMAs

# Store the distance matrix as fp16 on device (rel err ~5e-4, well inside
# the 2e-2 budget); host converts back to fp32. Halves the dominant HBM
# store traffic of this memory-bound kernel.
OUT_FP16 = True
WARM_PE = True

_cache = {}


def _build_nc(n_iters=1):
    from contextlib import ExitStack

    import concourse.bacc as bacc
    import concourse.tile as tile
    from concourse import mybir

    f32 = mybir.dt.float32
    f32r = mybir.dt.float32r
    f16 = mybir.dt.float16
    out_dt = f16 if OUT_FP16 else f32
    Act = mybir.ActivationFunctionType

    nc = bacc.Bacc("TRN2", target_bir_lowering=False, debug=False,
                   num_devices=N_CORES)
    # fp16 matmul operands: halves input HBM traffic, and the PE runs
    # 2-byte matmuls at full rate (1 cyc/row)
    xaT = nc.dram_tensor("xaT", [K_AUG, ROWS_PER_CORE], f16,
                         kind="ExternalInput")
    yaT = nc.dram_tensor("yaT", [K_AUG, N], f16, kind="ExternalInput")
    out = nc.dram_tensor("out", [ROWS_PER_CORE, N], out_dt,
                         kind="ExternalOutput")

    with tile.TileContext(nc) as tc, ExitStack() as ctx:
        singles = ctx.enter_context(tc.tile_pool(name="singles", bufs=1))
        mats = ctx.enter_context(tc.tile_pool(name="mats", bufs=2))
        mm_psum = ctx.enter_context(
            tc.tile_pool(name="mm_psum", bufs=4, space="PSUM"))
        outs = ctx.enter_context(tc.tile_pool(name="outs", bufs=6))

        dummy = singles.tile([128, 1], f32)
        warm_a = singles.tile([128, 128], f32)
        warm_b = singles.tile([128, 260], f32)

        for _it in range(n_iters):
            xa = mats.tile([K_AUG, ROWS_PER_CORE], f16, tag="xa")
            # ya halves as separate tiles so the first matmuls RAW-depend
            # only on the half they actually read
            yah = [mats.tile([K_AUG, N_HALF], f16, tag=f"ya{h}",
                             name=f"ya{h}") for h in range(2)]
            # queue split: ya half 0 on SP (HWDGE), xa + ya half 1 on Pool
            # (SWDGE) — three loads land concurrently
            nc.gpsimd.dma_start(out=xa, in_=xaT[:, :])
            nc.sync.dma_start(out=yah[0], in_=yaT[:, 0:N_HALF])
            nc.gpsimd.dma_start(out=yah[1], in_=yaT[:, N_HALF:N])

            if _it == 0:
                # preload the sqrt ACT table while the input DMAs fly
                nc.vector.memset(dummy, 1.0)
                nc.scalar.activation(out=dummy, in_=dummy, func=Act.Sqrt)
                if WARM_PE:
                    # keep the PE busy from t~1us so the HAM clock-gate
                    # ramp completes during the real matmul stream
                    nc.vector.memset(warm_a, 0.0)
                    nc.vector.memset(warm_b, 0.0)
                    wps = mm_psum.tile([M_TILE, N_HALF], f32, tag="mm",
                                       name="warm")
                    nc.tensor.matmul(wps[:, 0:260], lhsT=warm_a,
                                     rhs=warm_b, start=True, stop=True)

            # matmul order follows ya-half arrival: both m-blocks of the
            # first two q tiles, then both m-blocks of the last two.
            # psum/out tiles are per (m, half) chunk so the q2/q3 matmuls
            # have no false WAR dependency on the chunk-0 activations.
            for half in range(2):
                for m in range(N_MTILES):
                    lhsT = xa[:, m * M_TILE:(m + 1) * M_TILE]
                    ps = mm_psum.tile([M_TILE, N_HALF], f32, tag="mm",
                                      name=f"ps_m{m}h{half}")
                    for i in range(2):
                        nc.tensor.matmul(
                            ps[:, i * N_TILE:(i + 1) * N_TILE],
                            lhsT=lhsT,
                            rhs=yah[half][:, i * N_TILE:(i + 1) * N_TILE],
                            start=True, stop=True)
                    # psum = x.y - |x|^2/2 - |y|^2/2 = -d^2/2; sqrt the
                    # 1024-col chunk as soon as its two banks are filled
                    ot = outs.tile([M_TILE, N_HALF], out_dt, tag="ot",
                                   name=f"ot_m{m}h{half}")
                    nc.scalar.activation(out=ot, in_=ps,
                                         func=Act.Sqrt, scale=-2.0)
                    # store chunk on the SP HWDGE queue (HW A/B: cheaper
                    # than SWDGE stores, and keeps ACT free for sqrts)
                    nc.sync.dma_start(
                        out=out[m * M_TILE:(m + 1) * M_TILE,
                                half * N_HALF:(half + 1) * N_HALF],
                        in_=ot)

    nc.compile()
    return nc


def _make_runner(nc):
    """Cached jitted SPMD executor (mirrors bass2jax.run_bass_via_pjrt, but
    reuses one jax.jit wrapper so the NEFF is not re-loaded per call)."""
    import jax
    from jax.experimental.shard_map import shard_map
    from jax.sharding import Mesh, PartitionSpec

    from concourse import bass2jax, mybir

    bass2jax.install_neuronx_cc_hook()
    assert nc.dbg_addr is None

    partition_name = (nc.partition_id_tensor.name
                      if nc.partition_id_tensor else None)
    in_names, out_names, out_avals, zero_shapes = [], [], [], []
    for alloc in nc.m.functions[0].allocations:
        if not isinstance(alloc, mybir.MemoryLocationSet):
            continue
        name = alloc.memorylocations[0].name
        if alloc.kind == "ExternalInput":
            if name != partition_name:
                in_names.append(name)
        elif alloc.kind == "ExternalOutput":
            shape = tuple(alloc.tensor_shape)
            dtype = mybir.dt.np(alloc.dtype)
            out_names.append(name)
            out_avals.append(jax.core.ShapedArray(shape, dtype))
            zero_shapes.append((shape, dtype))
    n_params = len(in_names)
    n_outs = len(out_names)
    all_in_names = list(in_names + out_names)
    if partition_name is not None:
        all_in_names.append(partition_name)
    all_in_names = tuple(all_in_names)
    donate = tuple(range(n_params, n_params + n_outs))

    def _body(*args):
        operands = list(args)
        if partition_name is not None:
            operands.append(bass2jax.partition_id_tensor())
        outs = bass2jax._bass_exec_p.bind(
            *operands,
            out_avals=tuple(out_avals),
            in_names=all_in_names,
            out_names=tuple(out_names),
            lowering_input_output_aliases=(),
            sim_require_finite=True,
            sim_require_nnan=True,
            nc=nc,
        )
        return tuple(outs)

    devices = jax.devices()[:N_CORES]
    mesh = Mesh(np.asarray(devices), ("core",))
    sharded = jax.jit(
        shard_map(_body, mesh=mesh,
                  in_specs=(PartitionSpec("core"),) * (n_params + n_outs),
                  out_specs=(PartitionSpec("core"),) * n_outs,
                  check_rep=False),
        donate_argnums=donate, keep_unused=True)

    def run(in_maps):
        concat_in = [
            np.concatenate([np.asarray(m[name]) for m in in_maps], axis=0)
            for name in in_names
        ]
        concat_zeros = [
            np.zeros((N_CORES * s[0], *s[1:]), dt) for s, dt in zero_shapes
        ]
        out_arrs = sharded(*concat_in, *concat_zeros)
        return [
            {name: np.asarray(out_arrs[i]).reshape(
                N_CORES, *zero_shapes[i][0])[c]
             for i, name in enumerate(out_names)}
            for c in range(N_CORES)
        ]

    run.sharded = sharded
    run.in_names = in_names
    run.out_names = out_names
    run.zero_shapes = zero_shapes
    run.mesh = mesh
    return run


def _get_runner():
    if "run" not in _cache:
        _cache["run"] = _make_runner(_build_nc())
    return _cache["run"]


def _shard_inputs(x, y):
    """Host-side shard + relayout: per core, matmul-native operands.

    psum[i, j] = sum_k xaT[k, i] * yaT[k, j]
               = x_i . y_j - |y_j|^2/2 - |x_i|^2/2 = -d^2/2
    so out = sqrt(-2 * psum).
    """
    ya = np.empty((K_AUG, N), dtype=np.float16)
    ya[0:D, :] = y.T.astype(np.float16)
    ya[D, :] = (-0.5 * (y.astype(np.float64) ** 2).sum(1)).astype(np.float16)
    ya[D + 1, :] = 1.0
    ya = np.ascontiguousarray(ya)
    in_maps = []
    for c in range(N_CORES):
        xs = x[c * ROWS_PER_CORE:(c + 1) * ROWS_PER_CORE, :]
        xa = np.empty((K_AUG, ROWS_PER_CORE), dtype=np.float16)
        xa[0:D, :] = xs.T.astype(np.float16)
        xa[D, :] = 1.0
        xa[D + 1, :] = (-0.5 * (xs.astype(np.float64) ** 2).sum(1)
                        ).astype(np.float16)
        in_maps.append({
            "xaT": np.ascontiguousarray(xa),
            "yaT": ya,
        })
    return in_maps


def kernel(x, y, **_ignored):
    x = np.ascontiguousarray(np.asarray(x), dtype=np.float32)
    y = np.ascontiguousarray(np.asarray(y), dtype=np.float32)
    assert x.shape == (N, D) and y.shape == (N, D)

    run = _get_runner()
    results = run(_shard_inputs(x, y))
    full = np.concatenate([results[c]["out"] for c in range(N_CORES)],
                          axis=0)
    return np.ascontiguousarray(full.astype(np.float32))



# revision 50
# speedup vs baseline: 1.0239x; 1.0239x over previous
"""CDist kernel for Trainium2 (8 NeuronCores, SPMD data-parallel over x rows).

out[i, j] = sqrt(sum_d (x[i,d] - y[j,d])^2),  x: [2048, 64], y: [2048, 64].

Sharding: x rows split 8 ways (256 rows/core), y replicated. Host-side
prep re-lays the inputs into matmul-native [K, N] fp16 operands with both
norm terms folded into an augmented K=66 contraction:
  lhsT = xaT [66, 256]: rows 0..63 = x^T, row 64 = 1, row 65 = -|x_i|^2/2
  rhs  = yaT [66, 2048]: rows 0..63 = y^T, row 64 = -|y_j|^2/2, row 65 = 1
so psum = x.y - |x|^2/2 - |y|^2/2 = -d^2/2 and each [128, 1024] output
chunk is one ScalarE instruction: sqrt(-2 * psum), written as fp16 (the
rel-err budget is 2e-2; fp16 in/out costs ~8e-4 and halves the HBM
traffic that bounds this memory-regime problem).

Device pipeline per core (7 DMAs total; loads xa + ya-half-1 ride the
otherwise-idle Pool SWDGE queue, ya-half-0 and all stores the SP HWDGE
queue — HW-measured fastest split): load xa + two ya halves -> 8 fp16
matmuls ([128,512] each, ordered to chase the ya-half arrivals) -> 4
chunked sqrts on ACT (the serial ~4.2us floor) -> 4 chunked stores that
start as soon as their chunk's sqrt lands. PSUM and output tiles are
per-chunk so the tile-granularity dependency tracker introduces no
false WAR serialization; a small warm matmul keeps the PE HAM
clock-gate ramping during the input DMAs.

The O(N^2) work (matmuls, sqrts, the 8.4 MB of fp16 stores) runs
entirely on device; host marshaling is O(N*D) layout plus the final
fp16 -> fp32 cast of the returned array.
"""

import os

import numpy as np

# Persistent XLA/NEFF compile cache so repeated runs skip recompilation.
os.environ.setdefault("JAX_COMPILATION_CACHE_DIR", "/tmp/jax_comp_cache")

N = 2048
D = 64
N_CORES = 8
ROWS_PER_CORE = N // N_CORES  # 256

K_AUG = D + 2  # 66: data rows + (-|y|^2/2) row + (-|x|^2/2) row
M_TILE = 128
N_TILE = 512
N_MTILES = ROWS_PER_CORE // M_TILE  # 2
N_NTILES = N // N_TILE  # 4
N_HALF = N // 2  # 1024: ya arrives in two half-DMAs

# Store the distance matrix as fp16 on device (rel err ~5e-4, well inside
# the 2e-2 budget); host converts back to fp32. Halves the dominant HBM
# store traffic of this memory-bound kernel.
OUT_FP16 = True
WARM_PE = True

_cache = {}


def _build_nc(n_iters=1):
    from contextlib import ExitStack

    import concourse.bacc as bacc
    import concourse.tile as tile
    from concourse import mybir

    f32 = mybir.dt.float32
    f32r = mybir.dt.float32r
    f16 = mybir.dt.float16
    out_dt = f16 if OUT_FP16 else f32
    Act = mybir.ActivationFunctionType

    nc = bacc.Bacc("TRN2", target_bir_lowering=False, debug=False,
                   num_devices=N_CORES)
    # fp16 matmul operands: halves input HBM traffic, and the PE runs
    # 2-byte matmuls at full rate (1 cyc/row)
    xaT = nc.dram_tensor("xaT", [K_AUG, ROWS_PER_CORE], f16,
                         kind="ExternalInput")
    yaT = nc.dram_tensor("yaT", [K_AUG, N], f16, kind="ExternalInput")
    out = nc.dram_tensor("out", [ROWS_PER_CORE, N], out_dt,
                         kind="ExternalOutput")

    with tile.TileContext(nc) as tc, ExitStack() as ctx:
        singles = ctx.enter_context(tc.tile_pool(name="singles", bufs=1))
        mats = ctx.enter_context(tc.tile_pool(name="mats", bufs=2))
        mm_psum = ctx.enter_context(
            tc.tile_pool(name="mm_psum", bufs=4, space="PSUM"))
        outs = ctx.enter_context(tc.tile_pool(name="outs", bufs=6))

        dummy = singles.tile([128, 1], f32)
        warm_a = singles.tile([128, 128], f32)
        warm_b = singles.tile([128, 260], f32)

        for _it in range(n_iters):
            xa = mats.tile([K_AUG, ROWS_PER_CORE], f16, tag="xa")
            # ya halves as separate tiles so the first matmuls RAW-depend
            # only on the half they actually read
            yah = [mats.tile([K_AUG, N_HALF], f16, tag=f"ya{h}",
                             name=f"ya{h}") for h in range(2)]
            # queue split: ya half 0 on SP (HWDGE), xa + ya half 1 on Pool
            # (SWDGE) — three loads land concurrently
            nc.gpsimd.dma_start(out=xa, in_=xaT[:, :])
            nc.sync.dma_start(out=yah[0], in_=yaT[:, 0:N_HALF])
            nc.gpsimd.dma_start(out=yah[1], in_=yaT[:, N_HALF:N])

            if _it == 0:
                # preload the sqrt ACT table while the input DMAs fly
                nc.vector.memset(dummy, 1.0)
                nc.scalar.activation(out=dummy, in_=dummy, func=Act.Sqrt)
                if WARM_PE:
                    # keep the PE busy from t~1us so the HAM clock-gate
                    # ramp completes during the real matmul stream
                    nc.vector.memset(warm_a, 0.0)
                    nc.vector.memset(warm_b, 0.0)
                    wps = mm_psum.tile([M_TILE, N_HALF], f32, tag="mm",
                                       name="warm")
                    nc.tensor.matmul(wps[:, 0:260], lhsT=warm_a,
                                     rhs=warm_b, start=True, stop=True)

            # matmul order follows ya-half arrival: both m-blocks of the
            # first two q tiles, then both m-blocks of the last two.
            # psum/out tiles are per (m, half) chunk so the q2/q3 matmuls
            # have no false WAR dependency on the chunk-0 activations.
            for half in range(2):
                for m in range(N_MTILES):
                    lhsT = xa[:, m * M_TILE:(m + 1) * M_TILE]
                    ps = mm_psum.tile([M_TILE, N_HALF], f32, tag="mm",
                                      name=f"ps_m{m}h{half}")
                    for i in range(2):
                        nc.tensor.matmul(
                            ps[:, i * N_TILE:(i + 1) * N_TILE],
                            lhsT=lhsT,
                            rhs=yah[half][:, i * N_TILE:(i + 1) * N_TILE],
                            start=True, stop=True)
                    # psum = x.y - |x|^2/2 - |y|^2/2 = -d^2/2; sqrt the
                    # 1024-col chunk as soon as its two banks are filled
                    ot = outs.tile([M_TILE, N_HALF], out_dt, tag="ot",
                                   name=f"ot_m{m}h{half}")
                    nc.scalar.activation(out=ot, in_=ps,
                                         func=Act.Sqrt, scale=-2.0)
                    # store chunk on the SP HWDGE queue (HW A/B: cheaper
                    # than SWDGE stores, and keeps ACT free for sqrts)
                    nc.sync.dma_start(
                        out=out[m * M_TILE:(m + 1) * M_TILE,
                                half * N_HALF:(half + 1) * N_HALF],
                        in_=ot)

    nc.compile()
    return nc


def _make_runner(nc):
    """Cached jitted SPMD executor (mirrors bass2jax.run_bass_via_pjrt, but
    reuses one jax.jit wrapper so the NEFF is not re-loaded per call)."""
    import jax
    from jax.experimental.shard_map import shard_map
    from jax.sharding import Mesh, PartitionSpec

    from concourse import bass2jax, mybir

    bass2jax.install_neuronx_cc_hook()
    assert nc.dbg_addr is None

    partition_name = (nc.partition_id_tensor.name
                      if nc.partition_id_tensor else None)
    in_names, out_names, out_avals, zero_shapes = [], [], [], []
    for alloc in nc.m.functions[0].allocations:
        if not isinstance(alloc, mybir.MemoryLocationSet):
            continue
        name = alloc.memorylocations[0].name
        if alloc.kind == "ExternalInput":
            if name != partition_name:
                in_names.append(name)
        elif alloc.kind == "ExternalOutput":
            shape = tuple(alloc.tensor_shape)
            dtype = mybir.dt.np(alloc.dtype)
            out_names.append(name)
            out_avals.append(jax.core.ShapedArray(shape, dtype))
            zero_shapes.append((shape, dtype))
    n_params = len(in_names)
    n_outs = len(out_names)
    all_in_names = list(in_names + out_names)
    if partition_name is not None:
        all_in_names.append(partition_name)
    all_in_names = tuple(all_in_names)
    donate = tuple(range(n_params, n_params + n_outs))

    def _body(*args):
        operands = list(args)
        if partition_name is not None:
            operands.append(bass2jax.partition_id_tensor())
        outs = bass2jax._bass_exec_p.bind(
            *operands,
            out_avals=tuple(out_avals),
            in_names=all_in_names,
            out_names=tuple(out_names),
            lowering_input_output_aliases=(),
            sim_require_finite=True,
            sim_require_nnan=True,
            nc=nc,
        )
        return tuple(outs)

    devices = jax.devices()[:N_CORES]
    mesh = Mesh(np.asarray(devices), ("core",))
    sharded = jax.jit(
        shard_map(_body, mesh=mesh,
                  in_specs=(PartitionSpec("core"),) * (n_params + n_outs),
                  out_specs=(PartitionSpec("core"),) * n_outs,
                  check_rep=False),
        donate_argnums=donate, keep_unused=True)

    def run(in_maps):
        concat_in = [
            np.concatenate([np.asarray(m[name]) for m in in_maps], axis=0)
            for name in in_names
        ]
        concat_zeros = [
            np.zeros((N_CORES * s[0], *s[1:]), dt) for s, dt in zero_shapes
        ]
        out_arrs = sharded(*concat_in, *concat_zeros)
        return [
            {name: np.asarray(out_arrs[i]).reshape(
                N_CORES, *zero_shapes[i][0])[c]
             for i, name in enumerate(out_names)}
            for c in range(N_CORES)
        ]

    run.sharded = sharded
    run.in_names = in_names
    run.out_names = out_names
    run.zero_shapes = zero_shapes
    run.mesh = mesh
    return run


def _get_runner():
    if "run" not in _cache:
        _cache["run"] = _make_runner(_build_nc())
    return _cache["run"]


def _shard_inputs(x, y):
    """Host-side shard + relayout: per core, matmul-native operands.

    psum[i, j] = sum_k xaT[k, i] * yaT[k, j]
               = x_i . y_j - |y_j|^2/2 - |x_i|^2/2 = -d^2/2
    so out = sqrt(-2 * psum).
    """
    ya = np.empty((K_AUG, N), dtype=np.float16)
    ya[0:D, :] = y.T.astype(np.float16)
    ya[D, :] = (-0.5 * (y.astype(np.float64) ** 2).sum(1)).astype(np.float16)
    ya[D + 1, :] = 1.0
    ya = np.ascontiguousarray(ya)
    in_maps = []
    for c in range(N_CORES):
        xs = x[c * ROWS_PER_CORE:(c + 1) * ROWS_PER_CORE, :]
        xa = np.empty((K_AUG, ROWS_PER_CORE), dtype=np.float16)
        xa[0:D, :] = xs.T.astype(np.float16)
        xa[D, :] = 1.0
        xa[D + 1, :] = (-0.5 * (xs.astype(np.float64) ** 2).sum(1)
                        ).astype(np.float16)
        in_maps.append({
            "xaT": np.ascontiguousarray(xa),
            "yaT": ya,
        })
    return in_maps


def kernel(x, y, **_ignored):
    x = np.ascontiguousarray(np.asarray(x), dtype=np.float32)
    y = np.ascontiguousarray(np.asarray(y), dtype=np.float32)
    assert x.shape == (N, D) and y.shape == (N, D)

    run = _get_runner()
    results = run(_shard_inputs(x, y))
    full = np.concatenate([results[c]["out"] for c in range(N_CORES)],
                          axis=0)
    return np.ascontiguousarray(full.astype(np.float32))

